# revision 19
# baseline (speedup 1.0000x reference)
"""Trainium2 Bass kernel for nn_MaskedMessagePassingLayer.

Reference computation (B=32, N=64, D=256, E=32, DN=64, H=8, M=32):
    emb   = LN(embeddings) * mask                        [B,N,D]
    recv  = emb @ W_recv / sqrt2,  send = emb @ W_send   [B,N,H*M]
    rn    = nf @ W_recv_node,      sn = nf @ W_send_node [B,N,H*M]
    EP    = (edge * dm) @ W_edge                         [B,N,N,H*M]
    msgs  = tanh(recv_j + send_i) * sigmoid((rn_j+sn_i)/sqrt2) * EP * dm
    out   = LN_over_M( sum_i msgs )                      [B,N,H*M]

Strategy:
  - Data-parallel over B across 8 cores (4 batches per core).
  - Mask compaction on host: only active nodes (mask=1) are shipped/computed;
    masked pairs contribute exactly 0 (dm=0), masked rows are exactly 0.
    Batches are sorted by active count and dealt round-robin so the four
    per-core "slots" have similar K; SPMD requires identical shapes per slot.
  - On device, the [K,K,H*M] intermediate is built flash-style per
    (slot, hm-chunk, j-section) tile and reduced over senders immediately:
      * rank-2 broadcasts send_i + recv_j are computed ON THE PE as a
        matmul of the stacked [send;recv] against a 0/1 selector matrix
      * EP comes from a PE matmul of host-pretransposed edge features
      * ACT does tanh/sigmoid (PSUM source), DVE does the two products and
        the sender-axis reduction
  - All heavy matmuls use float32r (1 col/cycle); products use bf16 where it
    buys DVE 2x mode; accumulations stay f32.
"""

import math

import numpy as np
import ml_dtypes

B, N, D, E, DN, H, M = 32, 64, 256, 32, 64, 8, 32
HM = H * M  # 256
EPS = 1e-5
NCORES = 8
SLOTS = B // NCORES  # 4
WMAX = 1024  # max free width of a PSUM work tile; 2 x 512-col pieces
# (PE operand base partition must be 0/32/64; PSUM is 8 x 2KB banks)
SQRT2 = math.sqrt(2.0)

BF16 = np.dtype(ml_dtypes.bfloat16)

_prog_cache: dict = {}


def _sections(K: int):
    """Split the receiver axis into sections so Jsec*K <= WMAX."""
    jsec = max(1, min(K, WMAX // K))
    out = []
    j0 = 0
    while j0 < K:
        jw = min(jsec, K - j0)
        out.append((j0, jw))
        j0 += jw
    return out


def _build_selector(K: int) -> np.ndarray:
    """sel[r, j*K+i] = (r == i) + (r == K + j), shape [2K, K*K], bf16."""
    sel = np.zeros((2 * K, K, K), dtype=np.float32)
    idx = np.arange(K)
    sel[idx[:, None], np.arange(K)[None, :], idx[:, None]] = 1.0  # send rows
    sel[K + idx[:, None], idx[:, None], np.arange(K)[None, :]] = 1.0  # recv rows
    return sel.reshape(2 * K, K * K).astype(BF16)


def _build_program(Ks):
    import concourse.bass as bass
    from concourse import bacc, mybir
    import concourse.tile as tile
    from concourse.masks import make_identity

    f32 = mybir.dt.float32
    f32r = mybir.dt.float32r
    bf16 = mybir.dt.bfloat16
    AF = mybir.ActivationFunctionType
    OP = mybir.AluOpType
    AX = mybir.AxisListType

    nc = bacc.Bacc("TRN2", target_bir_lowering=False, debug=False,
                   num_devices=NCORES)

    # ---- DRAM tensors -------------------------------------------------
    w_edge_d = nc.dram_tensor("w_edge", [64, HM], f32r, kind="ExternalInput")
    w_recv_d = nc.dram_tensor("w_recv", [128, 2, HM], f32r, kind="ExternalInput")
    w_send_d = nc.dram_tensor("w_send", [128, 2, HM], f32r, kind="ExternalInput")
    w_rn_d = nc.dram_tensor("w_rn", [DN, HM], f32r, kind="ExternalInput")
    w_sn_d = nc.dram_tensor("w_sn", [DN, HM], f32r, kind="ExternalInput")
    emb_d, nft_d, sel_d, edge_d, out_d = [], [], [], [], []
    for s, K in enumerate(Ks):
        nsec = len(_sections(K))
        emb_d.append(nc.dram_tensor(f"emb{s}", [K, D], f32, kind="ExternalInput"))
        nft_d.append(nc.dram_tensor(f"nft{s}", [DN, 2 * K], f32r, kind="ExternalInput"))
        sel_d.append(nc.dram_tensor(f"sel{s}", [2 * K, K * K], bf16, kind="ExternalInput"))
        edge_d.append(nc.dram_tensor(f"edge{s}", [nsec, 64, 512], f32r, kind="ExternalInput"))
        out_d.append(nc.dram_tensor(f"out{s}", [K, HM], f32, kind="ExternalOutput"))

    with tile.TileContext(nc) as tc:
        with (
            tc.tile_pool(name="const", bufs=1) as const,
            tc.tile_pool(name="slotp", bufs=1) as slotp,
            tc.tile_pool(name="trans", bufs=2) as trans,
            tc.tile_pool(name="small", bufs=4) as small,
            tc.tile_pool(name="edgep", bufs=3) as edgep,
            tc.tile_pool(name="bfp", bufs=6) as bfp,
            tc.tile_pool(name="msgp", bufs=2) as msgp,
            tc.tile_pool(name="pbig", bufs=2, space="PSUM") as pbig,
            tc.tile_pool(name="pmid", bufs=1, space="PSUM") as pmid,
            tc.tile_pool(name="psml", bufs=1, space="PSUM") as psml,
        ):
            # ---- constants -------------------------------------------
            id_sb = const.tile([128, 128], f32)
            make_identity(nc, id_sb)
            eps_sb = const.tile([128, 1], f32)
            nc.vector.memset(eps_sb, EPS)
            w_edge_sb = const.tile([64, HM], f32r)
            nc.sync.dma_start(out=w_edge_sb, in_=w_edge_d[:])
            w_recv_sb = const.tile([128, 2, HM], f32r)
            nc.sync.dma_start(out=w_recv_sb, in_=w_recv_d[:])
            w_send_sb = const.tile([128, 2, HM], f32r)
            nc.sync.dma_start(out=w_send_sb, in_=w_send_d[:])
            w_rn_sb = const.tile([DN, HM], f32r)
            nc.sync.dma_start(out=w_rn_sb, in_=w_rn_d[:])
            w_sn_sb = const.tile([DN, HM], f32r)
            nc.sync.dma_start(out=w_sn_sb, in_=w_sn_d[:])

            # persistent per-slot tiles
            SR = [slotp.tile([128, HM], bf16, tag=f"sr{s}", name=f"sr{s}") for s in range(SLOTS)]
            NSR = [slotp.tile([128, HM], bf16, tag=f"nsr{s}", name=f"nsr{s}") for s in range(SLOTS)]
            sel_sb = [slotp.tile([2 * K, K * K], bf16, tag=f"sel{s}", name=f"selsb{s}")
                      for s, K in enumerate(Ks)]
            upd = [[slotp.tile([128, Ks[s]], f32, tag=f"upd{s}_{c}", name=f"upd{s}_{c}")
                    for c in range(2)] for s in range(SLOTS)]

            # ---- phase 1: LN(emb), projections, stacks ----------------
            for s, K in enumerate(Ks):
                nc.sync.dma_start(out=sel_sb[s], in_=sel_d[s][:])

                emb_sb = trans.tile([K, D], f32, tag="emb")
                nc.sync.dma_start(out=emb_sb, in_=emb_d[s][:])
                st6 = small.tile([K, 6], f32, tag="st6")
                nc.vector.bn_stats(out=st6, in_=emb_sb)
                mv = small.tile([K, 2], f32, tag="mv")
                nc.vector.bn_aggr(out=mv, in_=st6)
                std = small.tile([K, 1], f32, tag="std")
                nc.scalar.activation(out=std, in_=mv[:, 1:2], func=AF.Sqrt,
                                     bias=eps_sb[0:K], scale=1.0)
                rstd = small.tile([K, 1], f32, tag="rstd")
                nc.vector.reciprocal(out=rstd, in_=std)
                nc.vector.tensor_scalar(out=emb_sb, in0=emb_sb,
                                        scalar1=mv[:, 0:1], scalar2=rstd,
                                        op0=OP.subtract, op1=OP.mult)

                # transpose LN'd emb into [d, node] with a zero left half
                embT = trans.tile([128, 2, 2 * K], f32r, tag="embT")
                nc.vector.memset(embT[:].bitcast(f32), 0.0)
                for c in range(2):
                    tp = psml.tile([128, K], f32, tag="tp")
                    nc.tensor.transpose(out=tp, in_=emb_sb[:, bass.ts(c, 128)],
                                        identity=id_sb[0:K, 0:K])
                    nc.vector.tensor_copy(out=embT[:, c, K:2 * K], in_=tp)

                # SR stack: rows [0:K] = send, rows [K:2K] = recv
                srp = pmid.tile([2 * K, HM], f32, tag="srp")
                for c in range(2):
                    nc.tensor.matmul(out=srp, lhsT=embT[:, c, :],
                                     rhs=w_recv_sb[:, c, :],
                                     start=(c == 0), stop=False,
                                     skip_group_check=True)
                for c in range(2):
                    nc.tensor.matmul(out=srp[0:K, :],
                                     lhsT=embT[:, c, K:2 * K],
                                     rhs=w_send_sb[:, c, :],
                                     start=False, stop=(c == 1),
                                     skip_group_check=True)
                nc.vector.tensor_copy(out=SR[s][0:2 * K, :], in_=srp)

                # NSR stack: rows [0:K] = sn, rows [K:2K] = rn
                nft_sb = trans.tile([DN, 2 * K], f32r, tag="nft")
                nc.sync.dma_start(out=nft_sb, in_=nft_d[s][:])
                nsrp = pmid.tile([2 * K, HM], f32, tag="srp")
                nc.tensor.matmul(out=nsrp, lhsT=nft_sb[:],
                                 rhs=w_rn_sb[:],
                                 start=True, stop=False, skip_group_check=True)
                nc.tensor.matmul(out=nsrp[0:K, :],
                                 lhsT=nft_sb[:, K:2 * K],
                                 rhs=w_sn_sb[:],
                                 start=False, stop=True, skip_group_check=True)
                nc.vector.tensor_copy(out=NSR[s][0:2 * K, :], in_=nsrp)

            # ---- phase 2: flash loop over (slot, hm-chunk, j-section) --
            for s, K in enumerate(Ks):
                for isec, (j0, jw) in enumerate(_sections(K)):
                    W = jw * K
                    base = j0 * K
                    npieces = (W + 511) // 512
                    edge_sb = edgep.tile([64, 512], f32r, tag="edge")
                    nc.sync.dma_start(out=edge_sb, in_=edge_d[s][isec])
                    for c in range(2):
                        tin = pbig.tile([128, W], f32, tag="big")
                        for p in range(npieces):
                            c0, c1 = 512 * p, min(512 * (p + 1), W)
                            nc.tensor.matmul(
                                out=tin[:, c0:c1],
                                lhsT=SR[s][0:2 * K, bass.ts(c, 128)],
                                rhs=sel_sb[s][:, base + c0:base + c1],
                                start=True, stop=True)
                        t_sb = bfp.tile([128, W], bf16, tag="bf")
                        nc.scalar.activation(out=t_sb, in_=tin, func=AF.Tanh)

                        gin = pbig.tile([128, W], f32, tag="big")
                        for p in range(npieces):
                            c0, c1 = 512 * p, min(512 * (p + 1), W)
                            nc.tensor.matmul(
                                out=gin[:, c0:c1],
                                lhsT=NSR[s][0:2 * K, bass.ts(c, 128)],
                                rhs=sel_sb[s][:, base + c0:base + c1],
                                start=True, stop=True)
                        g_sb = bfp.tile([128, W], bf16, tag="bf")
                        nc.scalar.activation(out=g_sb, in_=gin, func=AF.Sigmoid)

                        ep = pbig.tile([128, W], f32, tag="big")
                        for p in range(npieces):
                            c0, c1 = 512 * p, min(512 * (p + 1), W)
                            nc.tensor.matmul(
                                out=ep[:, c0:c1],
                                lhsT=w_edge_sb[32 * p:32 * p + 32,
                                               bass.ts(c, 128)],
                                rhs=edge_sb[32 * p:32 * p + 32, 0:c1 - c0],
                                start=True, stop=True)

                        tg = bfp.tile([128, W], bf16, tag="bf")
                        nc.vector.tensor_mul(out=tg, in0=t_sb, in1=g_sb)
                        msg = msgp.tile([128, W], f32, tag="msg")
                        nc.vector.tensor_mul(out=msg, in0=tg, in1=ep)
                        nc.vector.reduce_sum(
                            out=upd[s][c][:, j0:j0 + jw],
                            in_=msg.rearrange("p (j i) -> p j i", i=K),
                            axis=AX.X)

            # ---- phase 3: transpose + LN over M, store ----------------
            for s, K in enumerate(Ks):
                updT = pmid.tile([K, HM], f32, tag="srp", name="updT")
                for c in range(2):
                    nc.tensor.transpose(out=updT[:, bass.ts(c, 128)],
                                        in_=upd[s][c][:, 0:K], identity=id_sb)
                s1 = small.tile([K, H], f32, tag="s1")
                nc.vector.reduce_sum(out=s1,
                                     in_=updT.rearrange("p (h m) -> p h m", m=M),
                                     axis=AX.X)
                sq = trans.tile([K, HM], f32, tag="sq")
                nc.scalar.activation(out=sq, in_=updT, func=AF.Square)
                s2 = small.tile([K, H], f32, tag="s2")
                nc.vector.reduce_sum(out=s2,
                                     in_=sq.rearrange("p (h m) -> p h m", m=M),
                                     axis=AX.X)
                mean = small.tile([K, H], f32, tag="mean")
                nc.vector.tensor_scalar_mul(out=mean, in0=s1, scalar1=1.0 / M)
                ex2 = small.tile([K, H], f32, tag="ex2")
                nc.vector.tensor_scalar_mul(out=ex2, in0=s2, scalar1=1.0 / M)
                m2 = small.tile([K, H], f32, tag="m2")
                nc.vector.tensor_mul(out=m2, in0=mean, in1=mean)
                var = small.tile([K, H], f32, tag="var")
                nc.vector.tensor_tensor(out=var, in0=ex2, in1=m2, op=OP.subtract)
                std8 = small.tile([K, H], f32, tag="std8")
                nc.scalar.activation(out=std8, in_=var, func=AF.Sqrt,
                                     bias=eps_sb[0:K], scale=1.0)
                rstd8 = small.tile([K, H], f32, tag="rstd8")
                nc.vector.reciprocal(out=rstd8, in_=std8)
                outln = trans.tile([K, HM], f32, tag="outln")
                for h in range(H):
                    nc.vector.tensor_scalar(
                        out=outln[:, h * M:(h + 1) * M],
                        in0=updT[:, h * M:(h + 1) * M],
                        scalar1=mean[:, h:h + 1], scalar2=rstd8[:, h:h + 1],
                        op0=OP.subtract, op1=OP.mult)
                nc.sync.dma_start(out=out_d[s][:], in_=outln)

    nc.compile()
    return nc


def _prep_host(inputs):
    """Compact by mask, deal batches to cores/slots, build all host arrays."""
    mask = np.asarray(inputs["mask"]).astype(bool)
    emb = np.asarray(inputs["embeddings"], dtype=np.float32)
    edge = np.asarray(inputs["edge_features"], dtype=np.float32)
    nf = np.asarray(inputs["node_features"], dtype=np.float32)

    acts = [np.flatnonzero(mask[b]) for b in range(B)]
    counts = np.array([len(a) for a in acts])
    order = np.argsort(-counts, kind="stable")

    Ks = []
    for s in range(SLOTS):
        grp = order[NCORES * s:NCORES * (s + 1)]
        kmax = int(counts[grp].max()) if len(grp) else 0
        K = max(4, kmax + (kmax % 2))  # even, >= 4
        Ks.append(min(N, K))
    Ks = tuple(Ks)

    # shared constants
    w_edge = np.ascontiguousarray(
        np.tile(np.asarray(inputs["W_edge"], np.float32).reshape(E, HM), (2, 1)))
    w_recv = (np.asarray(inputs["W_recv"], np.float32).reshape(D, HM) / SQRT2)
    w_send = (np.asarray(inputs["W_send"], np.float32).reshape(D, HM) / SQRT2)
    w_recv = np.ascontiguousarray(w_recv.reshape(2, 128, HM).transpose(1, 0, 2))
    w_send = np.ascontiguousarray(w_send.reshape(2, 128, HM).transpose(1, 0, 2))
    w_rn = np.ascontiguousarray(np.asarray(inputs["W_recv_node"], np.float32).reshape(DN, HM) / SQRT2)
    w_sn = np.ascontiguousarray(np.asarray(inputs["W_send_node"], np.float32).reshape(DN, HM) / SQRT2)
    sels = [_build_selector(K) for K in Ks]

    in_maps = []
    placement = []  # (core, slot) -> (b, kb, act)
    for core in range(NCORES):
        im = {"w_edge": w_edge, "w_recv": w_recv, "w_send": w_send,
              "w_rn": w_rn, "w_sn": w_sn}
        for s, K in enumerate(Ks):
            b = int(order[NCORES * s + core])
            act = acts[b]
            kb = len(act)
            placement.append((core, s, b, kb))

            e = np.zeros((K, D), np.float32)
            e[:kb] = emb[b, act]
            im[f"emb{s}"] = e

            nt = np.zeros((DN, 2 * K), np.float32)
            nt[:, K:K + kb] = nf[b, act].T
            im[f"nft{s}"] = nt

            im[f"sel{s}"] = sels[s]

            et = np.zeros((E, K, K), np.float32)
            if kb:
                et[:, :kb, :kb] = edge[b][np.ix_(act, act)].transpose(2, 1, 0)
            et = et.reshape(E, K * K)
            secs = _sections(K)
            e4 = np.zeros((len(secs), 64, 512), np.float32)
            for isec, (j0, jw) in enumerate(secs):
                Wsec = jw * K
                blk = et[:, j0 * K:j0 * K + Wsec]
                for p in range((Wsec + 511) // 512):
                    c0, c1 = 512 * p, min(512 * (p + 1), Wsec)
                    e4[isec, 32 * p:32 * p + 32, 0:c1 - c0] = blk[:, c0:c1]
            im[f"edge{s}"] = e4
        in_maps.append(im)
    return Ks, in_maps, placement


TRACE = False
LAST_EXEC_NS = None
LAST_RESULTS = None


def kernel(**inputs) -> np.ndarray:
    global LAST_EXEC_NS, LAST_RESULTS
    from concourse.bass_utils import run_bass_kernel_spmd

    Ks, in_maps, placement = _prep_host(inputs)
    if Ks not in _prog_cache:
        _prog_cache[Ks] = _build_program(Ks)
    nc = _prog_cache[Ks]

    res = run_bass_kernel_spmd(nc, in_maps, list(range(NCORES)), trace=TRACE)
    LAST_EXEC_NS = res.exec_time_ns
    LAST_RESULTS = res

    out = np.zeros((B, N, HM), np.float32)
    mask = np.asarray(inputs["mask"]).astype(bool)
    acts = [np.flatnonzero(mask[b]) for b in range(B)]
    for core, s, b, kb in placement:
        if kb:
            out[b, acts[b]] = res.results[core][f"out{s}"][:kb]
    return out


# revision 20
# speedup vs baseline: 1.1888x; 1.1888x over previous
"""Trainium2 Bass kernel for nn_MaskedMessagePassingLayer.

Reference computation (B=32, N=64, D=256, E=32, DN=64, H=8, M=32):
    emb   = LN(embeddings) * mask                        [B,N,D]
    recv  = emb @ W_recv / sqrt2,  send = emb @ W_send   [B,N,H*M]
    rn    = nf @ W_recv_node,      sn = nf @ W_send_node [B,N,H*M]
    EP    = (edge * dm) @ W_edge                         [B,N,N,H*M]
    msgs  = tanh(recv_j + send_i) * sigmoid((rn_j+sn_i)/sqrt2) * EP * dm
    out   = LN_over_M( sum_i msgs )                      [B,N,H*M]

Strategy:
  - Data-parallel over B across 8 cores (4 batches per core).
  - Mask compaction on host: only active nodes (mask=1) are shipped/computed;
    masked pairs contribute exactly 0 (dm=0), masked rows are exactly 0.
    Batches are sorted by active count and dealt round-robin so the four
    per-core "slots" have similar K; SPMD requires identical shapes per slot.
  - On device, the [K,K,H*M] intermediate is built flash-style per
    (slot, hm-chunk, j-section) tile and reduced over senders immediately:
      * rank-2 broadcasts send_i + recv_j are computed ON THE PE as a
        matmul of the stacked [send;recv] against a 0/1 selector matrix
      * EP comes from a PE matmul of host-pretransposed edge features
      * ACT does tanh/sigmoid (PSUM source), DVE does the two products and
        the sender-axis reduction
  - All heavy matmuls use float32r (1 col/cycle); products use bf16 where it
    buys DVE 2x mode; accumulations stay f32.
"""

import math

import numpy as np
import ml_dtypes

B, N, D, E, DN, H, M = 32, 64, 256, 32, 64, 8, 32
HM = H * M  # 256
EPS = 1e-5
NCORES = 8
SLOTS = B // NCORES  # 4
WMAX = 1024  # max free width of a PSUM work tile; 2 x 512-col pieces
# (PE operand base partition must be 0/32/64; PSUM is 8 x 2KB banks)
SQRT2 = math.sqrt(2.0)

BF16 = np.dtype(ml_dtypes.bfloat16)

_prog_cache: dict = {}


def _sections(K: int):
    """Split the receiver axis into balanced sections with Jsec*K <= WMAX."""
    nsec = -(-K // max(1, min(K, WMAX // K)))
    jsec = -(-K // nsec)
    out = []
    j0 = 0
    while j0 < K:
        jw = min(jsec, K - j0)
        out.append((j0, jw))
        j0 += jw
    return out


def _build_selector(K: int) -> np.ndarray:
    """sel[r, j*K+i] = (r == i) + (r == K + j), shape [2K, K*K], bf16."""
    sel = np.zeros((2 * K, K, K), dtype=np.float32)
    idx = np.arange(K)
    sel[idx[:, None], np.arange(K)[None, :], idx[:, None]] = 1.0  # send rows
    sel[K + idx[:, None], idx[:, None], np.arange(K)[None, :]] = 1.0  # recv rows
    return sel.reshape(2 * K, K * K).astype(BF16)


def _build_program(Ks):
    import concourse.bass as bass
    from concourse import bacc, mybir
    import concourse.tile as tile
    from concourse.masks import make_identity

    f32 = mybir.dt.float32
    f32r = mybir.dt.float32r
    bf16 = mybir.dt.bfloat16
    AF = mybir.ActivationFunctionType
    OP = mybir.AluOpType
    AX = mybir.AxisListType

    nc = bacc.Bacc("TRN2", target_bir_lowering=False, debug=False,
                   num_devices=NCORES)

    # ---- DRAM tensors -------------------------------------------------
    w_edge_d = nc.dram_tensor("w_edge", [64, HM], f32r, kind="ExternalInput")
    w_recv_d = nc.dram_tensor("w_recv", [128, 2, HM], f32r, kind="ExternalInput")
    w_send_d = nc.dram_tensor("w_send", [128, 2, HM], f32r, kind="ExternalInput")
    w_rn_d = nc.dram_tensor("w_rn", [DN, HM], f32r, kind="ExternalInput")
    w_sn_d = nc.dram_tensor("w_sn", [DN, HM], f32r, kind="ExternalInput")
    emb_d, nft_d, sel_d, edge_d, out_d = [], [], [], [], []
    for s, K in enumerate(Ks):
        nsec = len(_sections(K))
        emb_d.append(nc.dram_tensor(f"emb{s}", [K, D], f32, kind="ExternalInput"))
        nft_d.append(nc.dram_tensor(f"nft{s}", [DN, 2 * K], f32r, kind="ExternalInput"))
        sel_d.append(nc.dram_tensor(f"sel{s}", [2 * K, K * K], bf16, kind="ExternalInput"))
        edge_d.append(nc.dram_tensor(f"edge{s}", [nsec, 64, 512], f32r, kind="ExternalInput"))
        out_d.append(nc.dram_tensor(f"out{s}", [K, HM], f32, kind="ExternalOutput"))

    with tile.TileContext(nc) as tc:
        with (
            tc.tile_pool(name="const", bufs=1) as const,
            tc.tile_pool(name="slotp", bufs=1) as slotp,
            tc.tile_pool(name="trans", bufs=2) as trans,
            tc.tile_pool(name="small", bufs=4) as small,
            tc.tile_pool(name="edgep", bufs=3) as edgep,
            tc.tile_pool(name="bfp", bufs=6) as bfp,
            tc.tile_pool(name="msgp", bufs=2) as msgp,
            tc.tile_pool(name="pbig", bufs=3, space="PSUM") as pbig,
            tc.tile_pool(name="pmid", bufs=2, space="PSUM") as pmid,
        ):
            # ---- constants -------------------------------------------
            id_sb = const.tile([128, 128], f32)
            make_identity(nc, id_sb)
            eps_sb = const.tile([128, 1], f32)
            nc.vector.memset(eps_sb, EPS)
            w_edge_sb = const.tile([64, HM], f32r)
            nc.sync.dma_start(out=w_edge_sb, in_=w_edge_d[:])
            w_recv_sb = const.tile([128, 2, HM], f32r)
            nc.sync.dma_start(out=w_recv_sb, in_=w_recv_d[:])
            w_send_sb = const.tile([128, 2, HM], f32r)
            nc.sync.dma_start(out=w_send_sb, in_=w_send_d[:])
            w_rn_sb = const.tile([DN, HM], f32r)
            nc.sync.dma_start(out=w_rn_sb, in_=w_rn_d[:])
            w_sn_sb = const.tile([DN, HM], f32r)
            nc.sync.dma_start(out=w_sn_sb, in_=w_sn_d[:])

            # persistent per-slot tiles
            SR = [slotp.tile([128, HM], bf16, tag=f"sr{s}", name=f"sr{s}") for s in range(SLOTS)]
            NSR = [slotp.tile([128, HM], bf16, tag=f"nsr{s}", name=f"nsr{s}") for s in range(SLOTS)]
            sel_sb = [slotp.tile([2 * K, K * K], bf16, tag=f"sel{s}", name=f"selsb{s}")
                      for s, K in enumerate(Ks)]
            upd = [[slotp.tile([128, Ks[s]], f32, tag=f"upd{s}_{c}", name=f"upd{s}_{c}")
                    for c in range(2)] for s in range(SLOTS)]

            act_p1, act_p2, act_p3 = [], [], []
            # ---- phase 1: LN(emb), projections, stacks ----------------
            for s, K in enumerate(Ks):
                nc.sync.dma_start(out=sel_sb[s], in_=sel_d[s][:])

                emb_sb = trans.tile([K, D], f32, tag="emb")
                nc.sync.dma_start(out=emb_sb, in_=emb_d[s][:])
                st6 = small.tile([K, 6], f32, tag="st6")
                nc.vector.bn_stats(out=st6, in_=emb_sb)
                mv = small.tile([K, 2], f32, tag="mv")
                nc.vector.bn_aggr(out=mv, in_=st6)
                std = small.tile([K, 1], f32, tag="std")
                act_p1.append(nc.scalar.activation(
                    out=std, in_=mv[:, 1:2], func=AF.Sqrt,
                    bias=eps_sb[0:K], scale=1.0))
                rstd = small.tile([K, 1], f32, tag="rstd")
                nc.vector.reciprocal(out=rstd, in_=std)
                nc.vector.tensor_scalar(out=emb_sb, in0=emb_sb,
                                        scalar1=mv[:, 0:1], scalar2=rstd,
                                        op0=OP.subtract, op1=OP.mult)

                # transpose LN'd emb into [d, node] with a zero left half
                embT = trans.tile([128, 2, 2 * K], f32r, tag="embT")
                nc.vector.memset(embT[:].bitcast(f32), 0.0)
                for c in range(2):
                    tp = pmid.tile([128, K], f32, tag="srp", name="tp")
                    nc.tensor.transpose(out=tp, in_=emb_sb[:, bass.ts(c, 128)],
                                        identity=id_sb[0:K, 0:K])
                    nc.scalar.copy(out=embT[:, c, K:2 * K], in_=tp)

                # SR stack: rows [0:K] = send, rows [K:2K] = recv
                srp = pmid.tile([2 * K, HM], f32, tag="srp")
                for c in range(2):
                    nc.tensor.matmul(out=srp, lhsT=embT[:, c, :],
                                     rhs=w_recv_sb[:, c, :],
                                     start=(c == 0), stop=False,
                                     skip_group_check=True)
                for c in range(2):
                    nc.tensor.matmul(out=srp[0:K, :],
                                     lhsT=embT[:, c, K:2 * K],
                                     rhs=w_send_sb[:, c, :],
                                     start=False, stop=(c == 1),
                                     skip_group_check=True)
                nc.scalar.copy(out=SR[s][0:2 * K, :], in_=srp)

                # NSR stack: rows [0:K] = sn, rows [K:2K] = rn
                nft_sb = trans.tile([DN, 2 * K], f32r, tag="nft")
                nc.sync.dma_start(out=nft_sb, in_=nft_d[s][:])
                nsrp = pmid.tile([2 * K, HM], f32, tag="srp")
                nc.tensor.matmul(out=nsrp, lhsT=nft_sb[:],
                                 rhs=w_rn_sb[:],
                                 start=True, stop=False, skip_group_check=True)
                nc.tensor.matmul(out=nsrp[0:K, :],
                                 lhsT=nft_sb[:, K:2 * K],
                                 rhs=w_sn_sb[:],
                                 start=False, stop=True, skip_group_check=True)
                nc.scalar.copy(out=NSR[s][0:2 * K, :], in_=nsrp)

            # ---- phase 2: flash loop over (slot, hm-chunk, j-section) --
            for s, K in enumerate(Ks):
                for isec, (j0, jw) in enumerate(_sections(K)):
                    W = jw * K
                    base = j0 * K
                    npieces = (W + 511) // 512
                    edge_sb = edgep.tile([64, 512], f32r, tag="edge")
                    nc.sync.dma_start(out=edge_sb, in_=edge_d[s][isec])
                    for c in range(2):
                        tin = pbig.tile([128, W], f32, tag="big")
                        for p in range(npieces):
                            c0, c1 = 512 * p, min(512 * (p + 1), W)
                            nc.tensor.matmul(
                                out=tin[:, c0:c1],
                                lhsT=SR[s][0:2 * K, bass.ts(c, 128)],
                                rhs=sel_sb[s][:, base + c0:base + c1],
                                start=True, stop=True)
                        t_sb = bfp.tile([128, W], bf16, tag="bf")
                        act_p2.append(nc.scalar.activation(
                            out=t_sb, in_=tin, func=AF.Tanh))

                        gin = pbig.tile([128, W], f32, tag="big")
                        for p in range(npieces):
                            c0, c1 = 512 * p, min(512 * (p + 1), W)
                            nc.tensor.matmul(
                                out=gin[:, c0:c1],
                                lhsT=NSR[s][0:2 * K, bass.ts(c, 128)],
                                rhs=sel_sb[s][:, base + c0:base + c1],
                                start=True, stop=True)
                        g_sb = bfp.tile([128, W], bf16, tag="bf")
                        act_p2.append(nc.scalar.activation(
                            out=g_sb, in_=gin, func=AF.Sigmoid))

                        ep = pbig.tile([128, W], f32, tag="big")
                        for p in range(npieces):
                            c0, c1 = 512 * p, min(512 * (p + 1), W)
                            nc.tensor.matmul(
                                out=ep[:, c0:c1],
                                lhsT=w_edge_sb[32 * p:32 * p + 32,
                                               bass.ts(c, 128)],
                                rhs=edge_sb[32 * p:32 * p + 32, 0:c1 - c0],
                                start=True, stop=True)

                        tg = bfp.tile([128, W], bf16, tag="bf")
                        nc.vector.tensor_mul(out=tg, in0=t_sb, in1=g_sb)
                        msg = msgp.tile([128, W], f32, tag="msg")
                        nc.vector.tensor_mul(out=msg, in0=tg, in1=ep)
                        nc.vector.reduce_sum(
                            out=upd[s][c][:, j0:j0 + jw],
                            in_=msg.rearrange("p (j i) -> p j i", i=K),
                            axis=AX.X)

            # ---- phase 3: transpose + LN over M, store ----------------
            for s, K in enumerate(Ks):
                updT = pmid.tile([K, HM], f32, tag="srp", name="updT")
                for c in range(2):
                    nc.tensor.transpose(out=updT[:, bass.ts(c, 128)],
                                        in_=upd[s][c][:, 0:K], identity=id_sb)
                s1 = small.tile([K, H], f32, tag="s1")
                nc.vector.reduce_sum(out=s1,
                                     in_=updT.rearrange("p (h m) -> p h m", m=M),
                                     axis=AX.X)
                sq = trans.tile([K, HM], f32, tag="sq")
                nc.scalar.activation(out=sq, in_=updT, func=AF.Square)
                s2 = small.tile([K, H], f32, tag="s2")
                nc.vector.reduce_sum(out=s2,
                                     in_=sq.rearrange("p (h m) -> p h m", m=M),
                                     axis=AX.X)
                mean = small.tile([K, H], f32, tag="mean")
                nc.vector.tensor_scalar_mul(out=mean, in0=s1, scalar1=1.0 / M)
                ex2 = small.tile([K, H], f32, tag="ex2")
                nc.vector.tensor_scalar_mul(out=ex2, in0=s2, scalar1=1.0 / M)
                m2 = small.tile([K, H], f32, tag="m2")
                nc.vector.tensor_mul(out=m2, in0=mean, in1=mean)
                var = small.tile([K, H], f32, tag="var")
                nc.vector.tensor_tensor(out=var, in0=ex2, in1=m2, op=OP.subtract)
                std8 = small.tile([K, H], f32, tag="std8")
                act_p3.append(nc.scalar.activation(
                    out=std8, in_=var, func=AF.Sqrt,
                    bias=eps_sb[0:K], scale=1.0))
                rstd8 = small.tile([K, H], f32, tag="rstd8")
                nc.vector.reciprocal(out=rstd8, in_=std8)
                outln = trans.tile([K, HM], f32, tag="outln")
                mean_b = bass.AP(tensor=mean.tensor, offset=mean.offset,
                                 ap=[list(mean.ap[0]), [1, H], [0, M]])
                rstd_b = bass.AP(tensor=rstd8.tensor, offset=rstd8.offset,
                                 ap=[list(rstd8.ap[0]), [1, H], [0, M]])
                nc.vector.tensor_tensor(out=outln, in0=updT, in1=mean_b,
                                        op=OP.subtract)
                nc.vector.tensor_tensor(out=outln, in0=outln, in1=rstd_b,
                                        op=OP.mult)
                nc.sync.dma_start(out=out_d[s][:], in_=outln)

            from concourse.tile_rust import add_dep_helper
            for a in act_p2:
                for b in act_p1:
                    add_dep_helper(a.ins, b.ins, sync=False,
                                   reason="ACT table order: phase2 after phase1")
            for a in act_p3:
                for b in act_p2:
                    add_dep_helper(a.ins, b.ins, sync=False,
                                   reason="ACT table order: phase3 after phase2")

    nc.compile()
    return nc


def _prep_host(inputs):
    """Compact by mask, deal batches to cores/slots, build all host arrays."""
    mask = np.asarray(inputs["mask"]).astype(bool)
    emb = np.asarray(inputs["embeddings"], dtype=np.float32)
    edge = np.asarray(inputs["edge_features"], dtype=np.float32)
    nf = np.asarray(inputs["node_features"], dtype=np.float32)

    acts = [np.flatnonzero(mask[b]) for b in range(B)]
    counts = np.array([len(a) for a in acts])
    order = np.argsort(-counts, kind="stable")

    Ks = []
    for s in range(SLOTS):
        grp = order[NCORES * s:NCORES * (s + 1)]
        kmax = int(counts[grp].max()) if len(grp) else 0
        K = max(4, kmax + (kmax % 2))  # even, >= 4
        Ks.append(min(N, K))
    Ks = tuple(Ks)

    # shared constants
    w_edge = np.ascontiguousarray(
        np.tile(np.asarray(inputs["W_edge"], np.float32).reshape(E, HM), (2, 1)))
    w_recv = (np.asarray(inputs["W_recv"], np.float32).reshape(D, HM) / SQRT2)
    w_send = (np.asarray(inputs["W_send"], np.float32).reshape(D, HM) / SQRT2)
    w_recv = np.ascontiguousarray(w_recv.reshape(2, 128, HM).transpose(1, 0, 2))
    w_send = np.ascontiguousarray(w_send.reshape(2, 128, HM).transpose(1, 0, 2))
    w_rn = np.ascontiguousarray(np.asarray(inputs["W_recv_node"], np.float32).reshape(DN, HM) / SQRT2)
    w_sn = np.ascontiguousarray(np.asarray(inputs["W_send_node"], np.float32).reshape(DN, HM) / SQRT2)
    sels = [_build_selector(K) for K in Ks]

    in_maps = []
    placement = []  # (core, slot) -> (b, kb, act)
    for core in range(NCORES):
        im = {"w_edge": w_edge, "w_recv": w_recv, "w_send": w_send,
              "w_rn": w_rn, "w_sn": w_sn}
        for s, K in enumerate(Ks):
            b = int(order[NCORES * s + core])
            act = acts[b]
            kb = len(act)
            placement.append((core, s, b, kb))

            e = np.zeros((K, D), np.float32)
            e[:kb] = emb[b, act]
            im[f"emb{s}"] = e

            nt = np.zeros((DN, 2 * K), np.float32)
            nt[:, K:K + kb] = nf[b, act].T
            im[f"nft{s}"] = nt

            im[f"sel{s}"] = sels[s]

            et = np.zeros((E, K, K), np.float32)
            if kb:
                et[:, :kb, :kb] = edge[b][np.ix_(act, act)].transpose(2, 1, 0)
            et = et.reshape(E, K * K)
            secs = _sections(K)
            e4 = np.zeros((len(secs), 64, 512), np.float32)
            for isec, (j0, jw) in enumerate(secs):
                Wsec = jw * K
                blk = et[:, j0 * K:j0 * K + Wsec]
                for p in range((Wsec + 511) // 512):
                    c0, c1 = 512 * p, min(512 * (p + 1), Wsec)
                    e4[isec, 32 * p:32 * p + 32, 0:c1 - c0] = blk[:, c0:c1]
            im[f"edge{s}"] = e4
        in_maps.append(im)
    return Ks, in_maps, placement


TRACE = False
LAST_EXEC_NS = None
LAST_RESULTS = None


def kernel(**inputs) -> np.ndarray:
    global LAST_EXEC_NS, LAST_RESULTS
    from concourse.bass_utils import run_bass_kernel_spmd

    Ks, in_maps, placement = _prep_host(inputs)
    if Ks not in _prog_cache:
        _prog_cache[Ks] = _build_program(Ks)
    nc = _prog_cache[Ks]

    res = run_bass_kernel_spmd(nc, in_maps, list(range(NCORES)), trace=TRACE)
    LAST_EXEC_NS = res.exec_time_ns
    LAST_RESULTS = res

    out = np.zeros((B, N, HM), np.float32)
    mask = np.asarray(inputs["mask"]).astype(bool)
    acts = [np.flatnonzero(mask[b]) for b in range(B)]
    for core, s, b, kb in placement:
        if kb:
            out[b, acts[b]] = res.results[core][f"out{s}"][:kb]
    return out


# revision 22
# speedup vs baseline: 1.2317x; 1.0361x over previous
"""Trainium2 Bass kernel for nn_MaskedMessagePassingLayer.

Reference computation (B=32, N=64, D=256, E=32, DN=64, H=8, M=32):
    emb   = LN(embeddings) * mask                        [B,N,D]
    recv  = emb @ W_recv / sqrt2,  send = emb @ W_send   [B,N,H*M]
    rn    = nf @ W_recv_node,      sn = nf @ W_send_node [B,N,H*M]
    EP    = (edge * dm) @ W_edge                         [B,N,N,H*M]
    msgs  = tanh(recv_j + send_i) * sigmoid((rn_j+sn_i)/sqrt2) * EP * dm
    out   = LN_over_M( sum_i msgs )                      [B,N,H*M]

Strategy:
  - Data-parallel over B across 8 cores (4 batches per core).
  - Mask compaction on host: only active nodes (mask=1) are shipped/computed;
    masked pairs contribute exactly 0 (dm=0), masked rows are exactly 0.
    Batches are sorted by active count and dealt round-robin so the four
    per-core "slots" have similar K; SPMD requires identical shapes per slot.
  - On device, the [K,K,H*M] intermediate is built flash-style per
    (slot, hm-chunk, j-section) tile and reduced over senders immediately:
      * rank-2 broadcasts send_i + recv_j are computed ON THE PE as a
        matmul of the stacked [send;recv] against a 0/1 selector matrix
      * EP comes from a PE matmul of host-pretransposed edge features
      * ACT does tanh/sigmoid (PSUM source), DVE does the two products and
        the sender-axis reduction
  - All heavy matmuls use float32r (1 col/cycle); products use bf16 where it
    buys DVE 2x mode; accumulations stay f32.
"""

import math

import numpy as np
import ml_dtypes

B, N, D, E, DN, H, M = 32, 64, 256, 32, 64, 8, 32
HM = H * M  # 256
EPS = 1e-5
NCORES = 8
SLOTS = B // NCORES  # 4
WMAX = 1024  # max free width of a PSUM work tile; 2 x 512-col pieces
# (PE operand base partition must be 0/32/64; PSUM is 8 x 2KB banks)
SQRT2 = math.sqrt(2.0)

BF16 = np.dtype(ml_dtypes.bfloat16)

_prog_cache: dict = {}


def _sections(K: int):
    """Split the receiver axis into balanced sections with Jsec*K <= WMAX."""
    nsec = -(-K // max(1, min(K, WMAX // K)))
    jsec = -(-K // nsec)
    out = []
    j0 = 0
    while j0 < K:
        jw = min(jsec, K - j0)
        out.append((j0, jw))
        j0 += jw
    return out


def _build_selector(K: int) -> np.ndarray:
    """sel[r, j*K+i] = (r == i) + (r == K + j), shape [2K, K*K], bf16."""
    sel = np.zeros((2 * K, K, K), dtype=np.float32)
    idx = np.arange(K)
    sel[idx[:, None], np.arange(K)[None, :], idx[:, None]] = 1.0  # send rows
    sel[K + idx[:, None], idx[:, None], np.arange(K)[None, :]] = 1.0  # recv rows
    return sel.reshape(2 * K, K * K).astype(BF16)


def _build_program(Ks):
    import concourse.bass as bass
    from concourse import bacc, mybir
    import concourse.tile as tile
    from concourse.masks import make_identity

    f32 = mybir.dt.float32
    f32r = mybir.dt.float32r
    bf16 = mybir.dt.bfloat16
    AF = mybir.ActivationFunctionType
    OP = mybir.AluOpType
    AX = mybir.AxisListType

    nc = bacc.Bacc("TRN2", target_bir_lowering=False, debug=False,
                   num_devices=NCORES)

    # ---- DRAM tensors -------------------------------------------------
    w_edge_d = nc.dram_tensor("w_edge", [64, HM], f32r, kind="ExternalInput")
    w_recv_d = nc.dram_tensor("w_recv", [128, 2, HM], f32r, kind="ExternalInput")
    w_send_d = nc.dram_tensor("w_send", [128, 2, HM], f32r, kind="ExternalInput")
    w_rn_d = nc.dram_tensor("w_rn", [DN, HM], f32r, kind="ExternalInput")
    w_sn_d = nc.dram_tensor("w_sn", [DN, HM], f32r, kind="ExternalInput")
    emb_d, nft_d, sel_d, edge_d, out_d = [], [], [], [], []
    for s, K in enumerate(Ks):
        nsec = len(_sections(K))
        emb_d.append(nc.dram_tensor(f"emb{s}", [K, D], f32, kind="ExternalInput"))
        nft_d.append(nc.dram_tensor(f"nft{s}", [DN, 2 * K], f32r, kind="ExternalInput"))
        sel_d.append(nc.dram_tensor(f"sel{s}", [2 * K, K * K], bf16, kind="ExternalInput"))
        edge_d.append(nc.dram_tensor(f"edge{s}", [nsec, 64, 512], f32r, kind="ExternalInput"))
        out_d.append(nc.dram_tensor(f"out{s}", [K, HM], f32, kind="ExternalOutput"))

    with tile.TileContext(nc) as tc:
        with (
            tc.tile_pool(name="const", bufs=1) as const,
            tc.tile_pool(name="slotp", bufs=1) as slotp,
            tc.tile_pool(name="trans", bufs=3) as trans,
            tc.tile_pool(name="small", bufs=4) as small,
            tc.tile_pool(name="edgep", bufs=3) as edgep,
            tc.tile_pool(name="bfp", bufs=6) as bfp,
            tc.tile_pool(name="msgp", bufs=2) as msgp,
            tc.tile_pool(name="pbig", bufs=1, space="PSUM") as pbig,
            tc.tile_pool(name="pmid", bufs=1, space="PSUM") as pmid,
        ):
            # ---- constants -------------------------------------------
            id_sb = const.tile([128, 128], f32)
            make_identity(nc, id_sb)
            eps_sb = const.tile([128, 1], f32)
            nc.vector.memset(eps_sb, EPS)
            w_edge_sb = const.tile([64, HM], f32r)
            nc.sync.dma_start(out=w_edge_sb, in_=w_edge_d[:])
            w_recv_sb = const.tile([128, 2, HM], f32r)
            nc.sync.dma_start(out=w_recv_sb, in_=w_recv_d[:])
            w_send_sb = const.tile([128, 2, HM], f32r)
            nc.sync.dma_start(out=w_send_sb, in_=w_send_d[:])
            w_rn_sb = const.tile([DN, HM], f32r)
            nc.sync.dma_start(out=w_rn_sb, in_=w_rn_d[:])
            w_sn_sb = const.tile([DN, HM], f32r)
            nc.sync.dma_start(out=w_sn_sb, in_=w_sn_d[:])

            # persistent per-slot tiles
            SR = [slotp.tile([128, HM], bf16, tag=f"sr{s}", name=f"sr{s}") for s in range(SLOTS)]
            NSR = [slotp.tile([128, HM], bf16, tag=f"nsr{s}", name=f"nsr{s}") for s in range(SLOTS)]
            sel_sb = [slotp.tile([2 * K, K * K], bf16, tag=f"sel{s}", name=f"selsb{s}")
                      for s, K in enumerate(Ks)]
            upd = [[slotp.tile([128, Ks[s]], f32, tag=f"upd{s}_{c}", name=f"upd{s}_{c}")
                    for c in range(2)] for s in range(SLOTS)]

            act_p1, act_p2, act_p3 = [], [], []
            # ---- phase 1: LN(emb), projections, stacks ----------------
            for s, K in enumerate(Ks):
                nc.sync.dma_start(out=sel_sb[s], in_=sel_d[s][:])

                emb_sb = trans.tile([K, D], f32, tag="emb")
                nc.gpsimd.dma_start(out=emb_sb, in_=emb_d[s][:])
                st6 = small.tile([K, 6], f32, tag="st6")
                nc.vector.bn_stats(out=st6, in_=emb_sb)
                mv = small.tile([K, 2], f32, tag="mv")
                nc.vector.bn_aggr(out=mv, in_=st6)
                std = small.tile([K, 1], f32, tag="std")
                act_p1.append(nc.scalar.activation(
                    out=std, in_=mv[:, 1:2], func=AF.Sqrt,
                    bias=eps_sb[0:K], scale=1.0))
                rstd = small.tile([K, 1], f32, tag="rstd")
                nc.vector.reciprocal(out=rstd, in_=std)
                nc.vector.tensor_scalar(out=emb_sb, in0=emb_sb,
                                        scalar1=mv[:, 0:1], scalar2=rstd,
                                        op0=OP.subtract, op1=OP.mult)

                # transpose LN'd emb into [d, node] with a zero left half
                embT = trans.tile([128, 2, 2 * K], f32r, tag="embT")
                nc.vector.memset(embT[:].bitcast(f32), 0.0)
                for c in range(2):
                    tp = pmid.tile([128, K], f32, tag="srp", name="tp")
                    nc.tensor.transpose(out=tp, in_=emb_sb[:, bass.ts(c, 128)],
                                        identity=id_sb[0:K, 0:K])
                    nc.scalar.copy(out=embT[:, c, K:2 * K], in_=tp)

                # SR stack: rows [0:K] = send, rows [K:2K] = recv
                srp = pmid.tile([2 * K, HM], f32, tag="srp")
                for c in range(2):
                    nc.tensor.matmul(out=srp, lhsT=embT[:, c, :],
                                     rhs=w_recv_sb[:, c, :],
                                     start=(c == 0), stop=False,
                                     skip_group_check=True)
                for c in range(2):
                    nc.tensor.matmul(out=srp[0:K, :],
                                     lhsT=embT[:, c, K:2 * K],
                                     rhs=w_send_sb[:, c, :],
                                     start=False, stop=(c == 1),
                                     skip_group_check=True)
                nc.scalar.copy(out=SR[s][0:2 * K, :], in_=srp)

                # NSR stack: rows [0:K] = sn, rows [K:2K] = rn
                nft_sb = trans.tile([DN, 2 * K], f32r, tag="nft")
                nc.sync.dma_start(out=nft_sb, in_=nft_d[s][:])
                nsrp = pmid.tile([2 * K, HM], f32, tag="srp")
                nc.tensor.matmul(out=nsrp, lhsT=nft_sb[:],
                                 rhs=w_rn_sb[:],
                                 start=True, stop=False, skip_group_check=True)
                nc.tensor.matmul(out=nsrp[0:K, :],
                                 lhsT=nft_sb[:, K:2 * K],
                                 rhs=w_sn_sb[:],
                                 start=False, stop=True, skip_group_check=True)
                nc.scalar.copy(out=NSR[s][0:2 * K, :], in_=nsrp)

            # ---- phase 2: flash loop over (slot, hm-chunk, j-section) --
            for s, K in enumerate(Ks):
                for isec, (j0, jw) in enumerate(_sections(K)):
                    W = jw * K
                    base = j0 * K
                    npieces = (W + 511) // 512
                    edge_sb = edgep.tile([64, 512], f32r, tag="edge")
                    nc.gpsimd.dma_start(out=edge_sb, in_=edge_d[s][isec])
                    for c in range(2):
                        tin = pbig.tile([128, W], f32, tag="tin")
                        for p in range(npieces):
                            c0, c1 = 512 * p, min(512 * (p + 1), W)
                            nc.tensor.matmul(
                                out=tin[:, c0:c1],
                                lhsT=SR[s][0:2 * K, bass.ts(c, 128)],
                                rhs=sel_sb[s][:, base + c0:base + c1],
                                start=True, stop=True)
                        t_sb = bfp.tile([128, W], bf16, tag="bf")
                        act_p2.append(nc.scalar.activation(
                            out=t_sb, in_=tin, func=AF.Tanh))

                        gin = pbig.tile([128, W], f32, tag="gin")
                        for p in range(npieces):
                            c0, c1 = 512 * p, min(512 * (p + 1), W)
                            nc.tensor.matmul(
                                out=gin[:, c0:c1],
                                lhsT=NSR[s][0:2 * K, bass.ts(c, 128)],
                                rhs=sel_sb[s][:, base + c0:base + c1],
                                start=True, stop=True)
                        g_sb = bfp.tile([128, W], bf16, tag="bf")
                        act_p2.append(nc.scalar.activation(
                            out=g_sb, in_=gin, func=AF.Sigmoid))

                        ep = pbig.tile([128, W], f32, tag="ep")
                        for p in range(npieces):
                            c0, c1 = 512 * p, min(512 * (p + 1), W)
                            nc.tensor.matmul(
                                out=ep[:, c0:c1],
                                lhsT=w_edge_sb[32 * p:32 * p + 32,
                                               bass.ts(c, 128)],
                                rhs=edge_sb[32 * p:32 * p + 32, 0:c1 - c0],
                                start=True, stop=True)

                        tg = bfp.tile([128, W], bf16, tag="bf")
                        nc.vector.tensor_mul(out=tg, in0=t_sb, in1=g_sb)
                        msg = msgp.tile([128, W], f32, tag="msg")
                        nc.vector.tensor_mul(out=msg, in0=tg, in1=ep)
                        nc.vector.reduce_sum(
                            out=upd[s][c][:, j0:j0 + jw],
                            in_=msg.rearrange("p (j i) -> p j i", i=K),
                            axis=AX.X)

            # ---- phase 3: transpose + LN over M, store ----------------
            for s, K in enumerate(Ks):
                updT = pmid.tile([K, HM], f32, tag="srp", name="updT")
                for c in range(2):
                    nc.tensor.transpose(out=updT[:, bass.ts(c, 128)],
                                        in_=upd[s][c][:, 0:K], identity=id_sb)
                s1 = small.tile([K, H], f32, tag="s1")
                nc.vector.reduce_sum(out=s1,
                                     in_=updT.rearrange("p (h m) -> p h m", m=M),
                                     axis=AX.X)
                sq = trans.tile([K, HM], f32, tag="sq")
                nc.scalar.activation(out=sq, in_=updT, func=AF.Square)
                s2 = small.tile([K, H], f32, tag="s2")
                nc.vector.reduce_sum(out=s2,
                                     in_=sq.rearrange("p (h m) -> p h m", m=M),
                                     axis=AX.X)
                mean = small.tile([K, H], f32, tag="mean")
                nc.vector.tensor_scalar_mul(out=mean, in0=s1, scalar1=1.0 / M)
                ex2 = small.tile([K, H], f32, tag="ex2")
                nc.vector.tensor_scalar_mul(out=ex2, in0=s2, scalar1=1.0 / M)
                m2 = small.tile([K, H], f32, tag="m2")
                nc.vector.tensor_mul(out=m2, in0=mean, in1=mean)
                var = small.tile([K, H], f32, tag="var")
                nc.vector.tensor_tensor(out=var, in0=ex2, in1=m2, op=OP.subtract)
                std8 = small.tile([K, H], f32, tag="std8")
                act_p3.append(nc.scalar.activation(
                    out=std8, in_=var, func=AF.Sqrt,
                    bias=eps_sb[0:K], scale=1.0))
                rstd8 = small.tile([K, H], f32, tag="rstd8")
                nc.vector.reciprocal(out=rstd8, in_=std8)
                outln = trans.tile([K, HM], f32, tag="outln")
                mean_b = bass.AP(tensor=mean.tensor, offset=mean.offset,
                                 ap=[list(mean.ap[0]), [1, H], [0, M]])
                rstd_b = bass.AP(tensor=rstd8.tensor, offset=rstd8.offset,
                                 ap=[list(rstd8.ap[0]), [1, H], [0, M]])
                nc.vector.tensor_tensor(out=outln, in0=updT, in1=mean_b,
                                        op=OP.subtract)
                nc.vector.tensor_tensor(out=outln, in0=outln, in1=rstd_b,
                                        op=OP.mult)
                nc.gpsimd.dma_start(out=out_d[s][:], in_=outln)

            from concourse.tile_rust import add_dep_helper
            for a in act_p2:
                for b in act_p1:
                    add_dep_helper(a.ins, b.ins, sync=False,
                                   reason="ACT table order: phase2 after phase1")
            for a in act_p3:
                for b in act_p2:
                    add_dep_helper(a.ins, b.ins, sync=False,
                                   reason="ACT table order: phase3 after phase2")

    nc.compile()
    return nc


def _prep_host(inputs):
    """Compact by mask, deal batches to cores/slots, build all host arrays."""
    mask = np.asarray(inputs["mask"]).astype(bool)
    emb = np.asarray(inputs["embeddings"], dtype=np.float32)
    edge = np.asarray(inputs["edge_features"], dtype=np.float32)
    nf = np.asarray(inputs["node_features"], dtype=np.float32)

    acts = [np.flatnonzero(mask[b]) for b in range(B)]
    counts = np.array([len(a) for a in acts])
    order = np.argsort(-counts, kind="stable")

    Ks = []
    for s in range(SLOTS):
        grp = order[NCORES * s:NCORES * (s + 1)]
        kmax = int(counts[grp].max()) if len(grp) else 0
        K = max(4, kmax + (kmax % 2))  # even, >= 4
        Ks.append(min(N, K))
    Ks = tuple(Ks)

    # shared constants
    w_edge = np.ascontiguousarray(
        np.tile(np.asarray(inputs["W_edge"], np.float32).reshape(E, HM), (2, 1)))
    w_recv = (np.asarray(inputs["W_recv"], np.float32).reshape(D, HM) / SQRT2)
    w_send = (np.asarray(inputs["W_send"], np.float32).reshape(D, HM) / SQRT2)
    w_recv = np.ascontiguousarray(w_recv.reshape(2, 128, HM).transpose(1, 0, 2))
    w_send = np.ascontiguousarray(w_send.reshape(2, 128, HM).transpose(1, 0, 2))
    w_rn = np.ascontiguousarray(np.asarray(inputs["W_recv_node"], np.float32).reshape(DN, HM) / SQRT2)
    w_sn = np.ascontiguousarray(np.asarray(inputs["W_send_node"], np.float32).reshape(DN, HM) / SQRT2)
    sels = [_build_selector(K) for K in Ks]

    in_maps = []
    placement = []  # (core, slot) -> (b, kb, act)
    for core in range(NCORES):
        im = {"w_edge": w_edge, "w_recv": w_recv, "w_send": w_send,
              "w_rn": w_rn, "w_sn": w_sn}
        for s, K in enumerate(Ks):
            b = int(order[NCORES * s + core])
            act = acts[b]
            kb = len(act)
            placement.append((core, s, b, kb))

            e = np.zeros((K, D), np.float32)
            e[:kb] = emb[b, act]
            im[f"emb{s}"] = e

            nt = np.zeros((DN, 2 * K), np.float32)
            nt[:, K:K + kb] = nf[b, act].T
            im[f"nft{s}"] = nt

            im[f"sel{s}"] = sels[s]

            et = np.zeros((E, K, K), np.float32)
            if kb:
                et[:, :kb, :kb] = edge[b][np.ix_(act, act)].transpose(2, 1, 0)
            et = et.reshape(E, K * K)
            secs = _sections(K)
            e4 = np.zeros((len(secs), 64, 512), np.float32)
            for isec, (j0, jw) in enumerate(secs):
                Wsec = jw * K
                blk = et[:, j0 * K:j0 * K + Wsec]
                for p in range((Wsec + 511) // 512):
                    c0, c1 = 512 * p, min(512 * (p + 1), Wsec)
                    e4[isec, 32 * p:32 * p + 32, 0:c1 - c0] = blk[:, c0:c1]
            im[f"edge{s}"] = e4
        in_maps.append(im)
    return Ks, in_maps, placement


TRACE = False
LAST_EXEC_NS = None
LAST_RESULTS = None


def kernel(**inputs) -> np.ndarray:
    global LAST_EXEC_NS, LAST_RESULTS
    from concourse.bass_utils import run_bass_kernel_spmd

    Ks, in_maps, placement = _prep_host(inputs)
    if Ks not in _prog_cache:
        _prog_cache[Ks] = _build_program(Ks)
    nc = _prog_cache[Ks]

    res = run_bass_kernel_spmd(nc, in_maps, list(range(NCORES)), trace=TRACE)
    LAST_EXEC_NS = res.exec_time_ns
    LAST_RESULTS = res

    out = np.zeros((B, N, HM), np.float32)
    mask = np.asarray(inputs["mask"]).astype(bool)
    acts = [np.flatnonzero(mask[b]) for b in range(B)]
    for core, s, b, kb in placement:
        if kb:
            out[b, acts[b]] = res.results[core][f"out{s}"][:kb]
    return out
